# revision 26
# baseline (speedup 1.0000x reference)
"""Trainium2 Bass kernel for nn_MetricLoss (retrieval_knn).

Sharding: data-parallel, one point cloud (4096 points) per NeuronCore, 8 cores.
v3: x-sorted windows + label-folded count + host-side prep.
(~725 us HW vs 2182 us for the v2 full-row winnow.)

Per core (points sorted by x on host):
  - PE: per 128-row block, s[i,j] = 2*p_i.p_j - |p_j|^2 over a static
    1792-wide window of x-sorted columns (verified to contain every
    point's top-40 neighbors with K=40 tie margin), via a bf16
    triple-split matmul (27 contraction rows, host-computed splits).
    A second PSUM stream prepends 3 label rows (exactly cancelling for
    same-label pairs; C=32 keeps C*lab^2 bf16-exact), giving
    s'' = s - 32*(lab_i-lab_j)^2 for the same-label count.
  - DVE: 5 max8 + 4 match_replace rounds (round 1 straight from PSUM)
    -> sorted top-40; fused scalar_tensor_tensor selects the pos/neg
    ranked values; ONE find_index8 pass recovers both column indices.
    s_sb is double-buffered so the ACT copy never gates the next
    block's round-1 max8.
  - ACT: psum->sbuf copy of s, and a Sign pass over s'' with accum_out
    = the same-label count (threshold = midpoint of v36/v37).
  - per-block gpsimd indirect_dma_start (offsets straight from SBUF,
    no DRAM index bounce) fetches packed (featN, sigma, label)
    neighbor rows; single batched elementwise tail (a split tail gets
    hoisted by the scheduler into the block loop where it stalls on
    fresh gathers); dAA=dPP=dNN=1 since features are pre-normalized on
    host; host sums masked terms in float64.
  - pos_idx==neg_idx rows (w=0 in the reference) get neg rotated and a
    host kill mask so the single find pass never double-matches.
"""

import numpy as np
import ml_dtypes

from concourse import bacc, bass as cbass, mybir, tile
from concourse.bass_utils import run_bass_kernel_spmd

B = 8
P = 4096
D = 32
K = 36
NB = P // 128          # 32 row blocks
W = 1792               # static window width (multiple of 128; 3*512+256)
CLAB = 32.0            # label penalty coefficient (pow2: C*lab^2 exact in bf16)
NEG_INF = -3.0e38
VAR_PRIOR = 1.0 / 96.0
KL_SCALE = 1e-6

f32 = mybir.dt.float32
bf16 = mybir.dt.bfloat16
i32 = mybir.dt.int32
i16 = mybir.dt.int16
u32 = mybir.dt.uint32
AF = mybir.ActivationFunctionType
OP = mybir.AluOpType
AX = mybir.AxisListType
bfnp = ml_dtypes.bfloat16


def _win_off(b: int) -> int:
    return min(max(128 * b + 64 - W // 2, 0), P - W)


def build_program(debug: bool = False):
    nc = bacc.Bacc("TRN2", target_bir_lowering=False, debug=debug)

    Ml_d = nc.dram_tensor("Ml", [30, P], bf16, kind="ExternalInput")
    Mm_d = nc.dram_tensor("Mm", [30, P], bf16, kind="ExternalInput")
    Ml7_d = nc.dram_tensor("Ml7", [27, P], bf16, kind="ExternalInput")
    Mm7_d = nc.dram_tensor("Mm7", [27, P], bf16, kind="ExternalInput")
    labb_d = nc.dram_tensor("labf", [128, NB], f32, kind="ExternalInput")
    sigb_d = nc.dram_tensor("sigb", [128, NB], f32, kind="ExternalInput")
    posb_d = nc.dram_tensor("pos1f", [128, NB], f32, kind="ExternalInput")
    negb_d = nc.dram_tensor("neg1f", [128, NB], f32, kind="ExternalInput")
    iota40_d = nc.dram_tensor("iota40f", [128, 40], f32,
                              kind="ExternalInput")
    featb_d = nc.dram_tensor("featb", [128, NB, D], f32, kind="ExternalInput")
    killb_d = nc.dram_tensor("killb", [128, NB], f32, kind="ExternalInput")
    outv_d = nc.dram_tensor("outv", [128, 8 * NB], f32, kind="ExternalOutput")

    pt_d = nc.dram_tensor("ptab", [P, 64], f32)

    with tile.TileContext(nc) as tc:
        with (
            tc.tile_pool(name="const", bufs=1) as consts,
            tc.tile_pool(name="sb", bufs=2) as sb,
            tc.tile_pool(name="scrp", bufs=1) as scrp,
            tc.tile_pool(name="psA", bufs=1, space="PSUM") as psA,
            tc.tile_pool(name="psB", bufs=1, space="PSUM") as psB,
        ):
            # ================= prep =================
            Ml = consts.tile([30, P], bf16)
            Mm = consts.tile([30, P], bf16)
            Ml7 = consts.tile([27, P], bf16)
            Mm7 = consts.tile([27, P], bf16)
            nc.sync.dma_start(Ml7, Ml7_d.ap())
            nc.sync.dma_start(Mm7, Mm7_d.ap())
            nc.sync.dma_start(Ml, Ml_d.ap())
            nc.sync.dma_start(Mm, Mm_d.ap())

            labbf = consts.tile([128, NB], f32)
            sigb = consts.tile([128, NB], f32)
            pos1f = consts.tile([128, NB], f32)
            neg1f = consts.tile([128, NB], f32)
            featb = consts.tile([128, NB, D], f32)
            nc.sync.dma_start(labbf, labb_d.ap())
            nc.sync.dma_start(sigb, sigb_d.ap())
            nc.sync.dma_start(pos1f, posb_d.ap())
            nc.sync.dma_start(neg1f, negb_d.ap())
            nc.sync.dma_start(featb, featb_d.ap())
            killb = consts.tile([128, NB], f32)
            nc.sync.dma_start(killb, killb_d.ap())

            # packed gather table (cols 34..39 uninitialized, never read)
            pt_v = pt_d.ap().rearrange("(b p) f -> p b f", p=128)
            nc.sync.dma_start(pt_v[:, :, 0:D], featb)
            nc.sync.dma_start(pt_v[:, :, D:D + 1],
                              sigb.rearrange("p (b o) -> p b o", o=1))
            nc.sync.dma_start(pt_v[:, :, D + 1:D + 2],
                              labbf.rearrange("p (b o) -> p b o", o=1))

            iota40f = consts.tile([128, 40], f32)
            nc.sync.dma_start(iota40f, iota40_d.ap())
            b1e7 = consts.tile([128, 1], f32)
            b1e8 = consts.tile([128, 1], f32)
            nc.vector.memset(b1e7, 1e-7)
            nc.vector.memset(b1e8, 1e-8)

            vsum = consts.tile([128, NB], f32)
            accB = consts.tile([128, NB], f32)
            find_in = consts.tile([128, 8], f32)
            nc.vector.memset(find_in, NEG_INF)
            Gp = consts.tile([128, NB, 64], f32)
            Gn = consts.tile([128, NB, 64], f32)

            scr = scrp.tile([128, W], f32, tag="scr")
            dummy = scrp.tile([128, W], bf16, tag="dummy")
            V40 = scrp.tile([128, 40], f32, tag="V40")
            idx8 = scrp.tile([128, 8], u32, tag="idx8")

            # ================= block loop =================
            for b in range(NB):
                ob = _win_off(b)
                psumA = psA.tile([128, W], f32, tag="A")
                psumB = psB.tile([128, W], f32, tag="B")
                s_sb = sb.tile([128, W], f32, tag="s_sb")
                for o0, w0 in ((0, 512), (512, 512), (1024, 512), (1536, 256)):
                    nc.tensor.matmul(psumA[:, o0:o0 + w0],
                                     Ml7[:, 128 * b:128 * (b + 1)],
                                     Mm7[:, ob + o0:ob + o0 + w0],
                                     start=True, stop=True)
                for o0, w0 in ((0, 512), (512, 512), (1024, 512), (1536, 256)):
                    nc.tensor.matmul(psumB[:, o0:o0 + w0],
                                     Ml[:, 128 * b:128 * (b + 1)],
                                     Mm[:, ob + o0:ob + o0 + w0],
                                     start=True, stop=True)

                nc.scalar.activation(s_sb, psumA, AF.Copy)

                # sorted top-40: 5 max8 rounds, round 1 from PSUM
                nc.vector.max(out=V40[:, 0:8], in_=psumA)
                nc.vector.match_replace(out=scr, in_to_replace=V40[:, 0:8],
                                        in_values=psumA, imm_value=NEG_INF)
                for rnd in range(1, 5):
                    nc.vector.max(out=V40[:, 8 * rnd:8 * (rnd + 1)], in_=scr)
                    if rnd < 4:
                        nc.vector.match_replace(
                            out=scr, in_to_replace=V40[:, 8 * rnd:8 * (rnd + 1)],
                            in_values=scr, imm_value=NEG_INF)

                # threshold midpoint -> vsum; same-label count on ACT
                nc.vector.tensor_add(vsum[:, b:b + 1], V40[:, 35:36],
                                     V40[:, 36:37])
                nc.scalar.activation(dummy, psumB, AF.Sign, scale=-2.0,
                                     bias=vsum[:, b:b + 1],
                                     accum_out=accB[:, b:b + 1])

                # rank-select pos/neg values straight into find input
                scr40 = sb.tile([128, 40], f32, tag="scr40")
                nc.vector.scalar_tensor_tensor(
                    out=scr40, in0=iota40f, scalar=pos1f[:, b:b + 1], in1=V40,
                    op0=OP.is_equal, op1=OP.mult, accum_out=find_in[:, 0:1])
                scr40b = sb.tile([128, 40], f32, tag="scr40b")
                nc.vector.scalar_tensor_tensor(
                    out=scr40b, in0=iota40f, scalar=neg1f[:, b:b + 1], in1=V40,
                    op0=OP.is_equal, op1=OP.mult, accum_out=find_in[:, 1:2])

                # one pass recovers both column indices (local), add offset
                nc.vector.max_index(out=idx8, in_max=find_in, in_values=s_sb)
                jpn32 = sb.tile([128, 2], i32, tag="jpn")
                nc.vector.tensor_scalar(jpn32, idx8[:, 0:2],
                                        float(ob), 4095.0, op0=OP.add,
                                        op1=OP.min)

                # per-block indirect gather straight from SBUF offsets
                nc.gpsimd.indirect_dma_start(
                    out=Gp[:, b], out_offset=None, in_=pt_d.ap(),
                    in_offset=cbass.IndirectOffsetOnAxis(
                        ap=jpn32[:, 0:1], axis=0))
                nc.gpsimd.indirect_dma_start(
                    out=Gn[:, b], out_offset=None, in_=pt_d.ap(),
                    in_offset=cbass.IndirectOffsetOnAxis(
                        ap=jpn32[:, 1:2], axis=0))

            # ============== loss tail (split: blocks 0..23, 24..31) ==============
            import os
            dbg = bool(os.environ.get("KDBG"))

            def emit_tail(c0, c1):
                nb = c1 - c0
                cs = slice(c0, c1)
                sfx = f"_{c0}"
                cntf = consts.tile([128, nb], f32, name=f"cntf{sfx}")
                nc.vector.tensor_scalar(cntf, accB[:, cs], -0.5,
                                        W / 2.0 - 1.0,
                                        op0=OP.mult, op1=OP.add)

                prod = consts.tile([128, nb, D], f32, name=f"prod{sfx}")
                dAP = consts.tile([128, nb], f32, name=f"dAP{sfx}")
                dAN = consts.tile([128, nb], f32, name=f"dAN{sfx}")
                dPN = consts.tile([128, nb], f32, name=f"dPN{sfx}")
                GpF = Gp[:, cs, 0:D]
                GnF = Gn[:, cs, 0:D]
                for dst, u, v in ((dAP, featb[:, cs], GpF),
                                  (dAN, featb[:, cs], GnF), (dPN, GpF, GnF)):
                    nc.vector.tensor_mul(prod, u, v)
                    nc.vector.tensor_reduce(dst, prod, axis=AX.X, op=OP.add)

                vA = sigb[:, cs]
                vP = consts.tile([128, nb], f32, name=f"vP{sfx}")
                vN = consts.tile([128, nb], f32, name=f"vN{sfx}")
                labP = consts.tile([128, nb], f32, name=f"labP{sfx}")
                labN = consts.tile([128, nb], f32, name=f"labN{sfx}")
                nc.vector.tensor_copy(
                    vP, Gp[:, cs, D:D + 1].rearrange("p b o -> p (b o)"))
                nc.vector.tensor_copy(
                    vN, Gn[:, cs, D:D + 1].rearrange("p b o -> p (b o)"))
                nc.vector.tensor_copy(
                    labP, Gp[:, cs, D + 1:D + 2].rearrange("p b o -> p (b o)"))
                nc.vector.tensor_copy(
                    labN, Gn[:, cs, D + 1:D + 2].rearrange("p b o -> p (b o)"))

                t1 = consts.tile([128, nb], f32, name=f"t1{sfx}")
                t2 = consts.tile([128, nb], f32, name=f"t2{sfx}")
                t3 = consts.tile([128, nb], f32, name=f"t3{sfx}")
                w = consts.tile([128, nb], f32, name=f"w{sfx}")
                nc.vector.tensor_tensor(t1, labP, labbf[:, cs], op=OP.is_equal)
                nc.vector.tensor_tensor(t2, labN, labbf[:, cs],
                                        op=OP.not_equal)
                nc.vector.tensor_mul(w, t1, t2)
                nc.vector.tensor_scalar(t1, cntf, 0.5, None, op0=OP.is_ge)
                nc.vector.tensor_mul(w, w, t1)
                nc.vector.tensor_scalar(t1, cntf, K - 1.5, None, op0=OP.is_le)
                nc.vector.tensor_mul(w, w, t1)
                nc.vector.tensor_mul(w, w, killb[:, cs])

                # mu = D*(vP - vN) - 2*(dAP - dAN)     (dPP = dNN = 1)
                mu = consts.tile([128, nb], f32, name=f"mu{sfx}")
                nc.vector.tensor_sub(t1, vP, vN)
                nc.vector.tensor_sub(t2, dAP, dAN)
                nc.vector.tensor_scalar_mul(t1, t1, float(D))
                nc.vector.scalar_tensor_tensor(
                    out=mu, in0=t2, scalar=-2.0, in1=t1,
                    op0=OP.mult, op1=OP.add)

                # sum_d T = D*vX^2 + (4 + 2D*vA - 4*dAX)*vX + 2*vA (dXX=dAA=1)
                def sT(out, vX, dAX):
                    nc.vector.tensor_scalar(t1, vA, 2.0 * D, 4.0,
                                            op0=OP.mult, op1=OP.add)
                    nc.vector.scalar_tensor_tensor(
                        out=t1, in0=dAX, scalar=-4.0, in1=t1,
                        op0=OP.mult, op1=OP.add)
                    nc.vector.tensor_mul(t1, t1, vX)
                    nc.vector.scalar_tensor_tensor(
                        out=t1, in0=vA, scalar=2.0, in1=t1,
                        op0=OP.mult, op1=OP.add)
                    nc.vector.scalar_tensor_tensor(
                        out=out, in0=vX, scalar=float(D), in1=vX,
                        op0=OP.mult, op1=OP.mult)
                    nc.vector.tensor_add(out, out, t1)

                sigma2 = consts.tile([128, nb], f32, name=f"sigma2{sfx}")
                sT(t2, vP, dAP)
                sT(t3, vN, dAN)
                nc.vector.tensor_add(sigma2, t2, t3)
                nc.vector.tensor_mul(t1, vA, dPN)
                nc.vector.scalar_tensor_tensor(
                    out=sigma2, in0=t1, scalar=-4.0, in1=sigma2,
                    op0=OP.mult, op1=OP.add)
                nc.vector.tensor_scalar_mul(sigma2, sigma2, 2.0)
                nc.vector.tensor_scalar_max(sigma2, sigma2, 0.0)

                sig = consts.tile([128, nb], f32, name=f"sig{sfx}")
                nc.scalar.activation(sig, sigma2, AF.Sqrt, bias=b1e7)
                nc.vector.tensor_scalar(t1, sig, 1e-8, float(np.sqrt(2.0)),
                                        op0=OP.add, op1=OP.mult)
                nc.vector.reciprocal(t2, t1)
                nc.vector.tensor_mul(t1, mu, t2)
                probs = consts.tile([128, nb], f32, name=f"probs{sfx}")
                nc.scalar.activation(probs, t1, AF.Erf, scale=-1.0)
                nc.vector.tensor_scalar(probs, probs, 0.5, 0.5,
                                        op0=OP.mult, op1=OP.add)
                nll = consts.tile([128, nb], f32, name=f"nll{sfx}")
                nc.scalar.activation(nll, probs, AF.Ln, bias=b1e8)
                nc.vector.tensor_scalar_mul(nll, nll, -1.0)

                # kl = (D/2)/VP*(vA+vP+vN) + const - (D/2)*ln(vA*vP*vN)
                kl = consts.tile([128, nb], f32, name=f"kl{sfx}")
                nc.vector.tensor_add(t1, vA, vP)
                nc.vector.tensor_add(t1, t1, vN)
                nc.vector.tensor_mul(t2, vA, vP)
                nc.vector.tensor_mul(t2, t2, vN)
                lnv = consts.tile([128, nb], f32, name=f"lnv{sfx}")
                nc.scalar.activation(lnv, t2, AF.Ln)
                kconst = 3.0 * (0.5 / VAR_PRIOR - D / 2.0
                                + (D / 2.0) * float(np.log(VAR_PRIOR)))
                nc.vector.tensor_scalar(t1, t1, 0.5 * D / VAR_PRIOR, kconst,
                                        op0=OP.mult, op1=OP.add)
                nc.vector.scalar_tensor_tensor(
                    out=kl, in0=lnv, scalar=-0.5 * D, in1=t1,
                    op0=OP.mult, op1=OP.add)

                rows = (w, nll, probs, mu, sig, kl, vsum[:, cs],
                        accB[:, cs]) if dbg else \
                    (w, nll, probs, mu, sig, kl, cntf, cntf)
                for qi, rsrc in enumerate(rows):
                    dst = outv_d.ap()[:, qi * NB + c0:qi * NB + c1]
                    if qi in (0, 6, 7):
                        nc.sync.dma_start(dst, rsrc)
                    else:
                        ot = consts.tile([128, nb], f32, name=f"o{qi}{sfx}")
                        nc.vector.tensor_mul(ot, rsrc, w)
                        nc.sync.dma_start(dst, ot)

            emit_tail(0, NB)

    nc.compile()
    return nc


_prog = None


def _get_prog():
    global _prog
    if _prog is None:
        _prog = build_program()
    return _prog


def _bf(x):
    return x.astype(bfnp)


def _f(x):
    return x.astype(np.float32)


def _build_M(pts, lab):
    """Host-side bf16 triple-split M matrices [30, P] (lhs, mov)."""
    x = np.ascontiguousarray(pts.T).astype(np.float32)      # [3, P]
    xh = _bf(x)
    res = x - _f(xh)
    xm = _bf(res)
    xl = _bf(res - _f(xm))
    nsq = -(x * x)
    nqh = _bf(nsq)
    nqr = nsq - _f(nqh)
    nqm = _bf(nqr)
    nql = _bf(nqr - _f(nqm))
    x2, x2b, x2c = _bf(2.0 * _f(xh)), _bf(2.0 * _f(xm)), _bf(2.0 * _f(xl))
    ones = np.ones((3, P), dtype=bfnp)
    labf = lab.astype(np.float32)

    Ml = np.zeros((30, P), dtype=bfnp)
    Mm = np.zeros((30, P), dtype=bfnp)
    # label penalty rows first: exact 0 for same-label pairs
    Ml[0] = _bf(-CLAB * labf * labf)
    Mm[0] = ones[0]
    Ml[1] = _bf(labf)
    Mm[1] = _bf(2.0 * CLAB * labf)
    Ml[2] = ones[0]
    Mm[2] = _bf(-CLAB * labf * labf)
    # s rows (baseline ordering), shifted by 3
    Ml[3:6], Mm[3:6] = x2b, xm          # mm
    Ml[6:9], Mm[6:9] = x2, xl           # hl
    Ml[9:12], Mm[9:12] = x2c, xh        # lh
    Ml[12:15], Mm[12:15] = ones, nql    # ql
    Ml[15:18], Mm[15:18] = x2, xm       # hm
    Ml[18:21], Mm[18:21] = x2b, xh      # mh
    Ml[21:24], Mm[21:24] = ones, nqm    # qm
    for c in range(3):
        Ml[24 + 2 * c], Mm[24 + 2 * c] = x2[c], xh[c]       # hh
        Ml[25 + 2 * c], Mm[25 + 2 * c] = ones[0], nqh[c]    # qh
    return Ml, Mm


def per_core_inputs(feature, sigma, xyz, label, pos_idx, neg_idx, c):
    lo, hi = c * P, (c + 1) * P
    pts = xyz[lo:hi, 1:4].astype(np.float64)
    order = np.argsort(pts[:, 0], kind='stable')
    pts = pts[order]
    lab = label[lo:hi, 0].astype(np.int32)[order]
    sig = sigma[lo:hi, 0].astype(np.float32)[order]
    pos = pos_idx[lo:hi].astype(np.int32)[order]
    neg = neg_idx[lo:hi].astype(np.int32)[order]
    same = pos == neg
    neg = np.where(same, (neg + 1) % (K - 1), neg).astype(np.int32)
    kill = (1.0 - same).astype(np.float32)
    feat = feature[lo:hi].astype(np.float64)[order]
    featN = (feat / np.linalg.norm(feat, axis=1, keepdims=True)).astype(
        np.float32)
    Ml, Mm = _build_M(pts.astype(np.float32), lab)
    return {
        "Ml": Ml,
        "Mm": Mm,
        "Ml7": np.ascontiguousarray(Ml[3:30]),
        "Mm7": np.ascontiguousarray(Mm[3:30]),
        "labf": np.ascontiguousarray(lab.astype(np.float32)
                                     .reshape(NB, 128).T),
        "sigb": np.ascontiguousarray(sig.reshape(NB, 128).T),
        "pos1f": np.ascontiguousarray((pos + 1).astype(np.float32)
                                      .reshape(NB, 128).T),
        "neg1f": np.ascontiguousarray((neg + 1).astype(np.float32)
                                      .reshape(NB, 128).T),
        "iota40f": np.tile(np.arange(40, dtype=np.float32), (128, 1)),
        "featb": np.ascontiguousarray(
            featN.reshape(NB, 128, D).transpose(1, 0, 2)),
        "killb": np.ascontiguousarray(kill.reshape(NB, 128).T),
    }


def unpack_rows(res):
    return np.concatenate(
        [r["outv"].astype(np.float64).reshape(128, 8, NB)
         .transpose(1, 2, 0).reshape(8, P) for r in res.results], axis=1)


def finalize(rows):
    ws = max(rows[0].sum(), 1.0)
    nll_m, probs_m, mu_m, sig_m, kl_m = (rows[i].sum() / ws
                                         for i in range(1, 6))
    loss = nll_m + KL_SCALE * kl_m
    return (np.float32(loss), np.float32(probs_m), np.float32(mu_m),
            np.float32(sig_m))


def kernel(feature, sigma, xyz, label, pos_idx, neg_idx):
    nc = _get_prog()
    in_maps = [
        per_core_inputs(feature, sigma, xyz, label, pos_idx, neg_idx, c)
        for c in range(B)
    ]
    res = run_bass_kernel_spmd(nc, in_maps, core_ids=list(range(B)))
    return finalize(unpack_rows(res))


# revision 27
# speedup vs baseline: 1.0044x; 1.0044x over previous
"""Trainium2 Bass kernel for nn_MetricLoss (retrieval_knn).

Sharding: data-parallel, one point cloud (4096 points) per NeuronCore, 8 cores.
v3: x-sorted windows + label-folded count + host-side prep.
(~725 us HW vs 2182 us for the v2 full-row winnow.)

Per core (points sorted by x on host):
  - PE: per 128-row block, s[i,j] = 2*p_i.p_j - |p_j|^2 over a static
    1792-wide window of x-sorted columns (verified to contain every
    point's top-40 neighbors with K=40 tie margin), via a bf16
    triple-split matmul (27 contraction rows, host-computed splits).
    A second PSUM stream prepends 3 label rows (exactly cancelling for
    same-label pairs; C=32 keeps C*lab^2 bf16-exact), giving
    s'' = s - 32*(lab_i-lab_j)^2 for the same-label count.
  - DVE: 5 max8 + 4 match_replace rounds (round 1 straight from PSUM)
    -> sorted top-40; fused scalar_tensor_tensor selects the pos/neg
    ranked values; ONE find_index8 pass recovers both column indices.
    s_sb is double-buffered so the ACT copy never gates the next
    block's round-1 max8.
  - ACT: psum->sbuf copy of s, and a Sign pass over s'' with accum_out
    = the same-label count (threshold = midpoint of v36/v37).
  - per-block gpsimd indirect_dma_start (offsets straight from SBUF,
    no DRAM index bounce) fetches packed (featN, sigma, label)
    neighbor rows; single batched elementwise tail (a split tail gets
    hoisted by the scheduler into the block loop where it stalls on
    fresh gathers); dAA=dPP=dNN=1 since features are pre-normalized on
    host; host sums masked terms in float64.
  - pos_idx==neg_idx rows (w=0 in the reference) get neg rotated and a
    host kill mask so the single find pass never double-matches.
"""

import numpy as np
import ml_dtypes

from concourse import bacc, bass as cbass, mybir, tile
from concourse.bass_utils import run_bass_kernel_spmd

B = 8
P = 4096
D = 32
K = 36
NB = P // 128          # 32 row blocks
W = 1792               # static window width (multiple of 128; 3*512+256)
CLAB = 32.0            # label penalty coefficient (pow2: C*lab^2 exact in bf16)
NEG_INF = -3.0e38
VAR_PRIOR = 1.0 / 96.0
KL_SCALE = 1e-6

f32 = mybir.dt.float32
bf16 = mybir.dt.bfloat16
i32 = mybir.dt.int32
i16 = mybir.dt.int16
u32 = mybir.dt.uint32
AF = mybir.ActivationFunctionType
OP = mybir.AluOpType
AX = mybir.AxisListType
bfnp = ml_dtypes.bfloat16


def _win_off(b: int) -> int:
    return min(max(128 * b + 64 - W // 2, 0), P - W)


def build_program(debug: bool = False):
    nc = bacc.Bacc("TRN2", target_bir_lowering=False, debug=debug)

    Ml_d = nc.dram_tensor("Ml", [30, P], bf16, kind="ExternalInput")
    Mm_d = nc.dram_tensor("Mm", [30, P], bf16, kind="ExternalInput")
    Ml7_d = nc.dram_tensor("Ml7", [27, P], bf16, kind="ExternalInput")
    Mm7_d = nc.dram_tensor("Mm7", [27, P], bf16, kind="ExternalInput")
    labb_d = nc.dram_tensor("labf", [128, NB], f32, kind="ExternalInput")
    sigb_d = nc.dram_tensor("sigb", [128, NB], f32, kind="ExternalInput")
    posb_d = nc.dram_tensor("pos1f", [128, NB], f32, kind="ExternalInput")
    negb_d = nc.dram_tensor("neg1f", [128, NB], f32, kind="ExternalInput")
    iota40_d = nc.dram_tensor("iota40f", [128, 40], f32,
                              kind="ExternalInput")
    featb_d = nc.dram_tensor("featb", [128, NB, D], f32, kind="ExternalInput")
    killb_d = nc.dram_tensor("killb", [128, NB], f32, kind="ExternalInput")
    outv_d = nc.dram_tensor("outv", [128, 8 * NB], f32, kind="ExternalOutput")

    pt_d = nc.dram_tensor("ptab", [P, 64], f32)

    with tile.TileContext(nc) as tc:
        with (
            tc.tile_pool(name="const", bufs=1) as consts,
            tc.tile_pool(name="sb", bufs=2) as sb,
            tc.tile_pool(name="scrp", bufs=1) as scrp,
            tc.tile_pool(name="psA", bufs=1, space="PSUM") as psA,
            tc.tile_pool(name="psB", bufs=1, space="PSUM") as psB,
        ):
            # ================= prep =================
            Ml = consts.tile([30, P], bf16)
            Mm = consts.tile([30, P], bf16)
            Ml7 = consts.tile([27, P], bf16)
            Mm7 = consts.tile([27, P], bf16)
            nc.sync.dma_start(Ml7[:, 0:128], Ml7_d.ap()[:, 0:128])
            nc.sync.dma_start(Mm7[:, 0:W], Mm7_d.ap()[:, 0:W])
            nc.sync.dma_start(Ml7[:, 128:], Ml7_d.ap()[:, 128:])
            nc.sync.dma_start(Mm7[:, W:], Mm7_d.ap()[:, W:])
            nc.sync.dma_start(Ml, Ml_d.ap())
            nc.sync.dma_start(Mm, Mm_d.ap())

            labbf = consts.tile([128, NB], f32)
            sigb = consts.tile([128, NB], f32)
            pos1f = consts.tile([128, NB], f32)
            neg1f = consts.tile([128, NB], f32)
            featb = consts.tile([128, NB, D], f32)
            nc.sync.dma_start(labbf, labb_d.ap())
            nc.sync.dma_start(sigb, sigb_d.ap())
            nc.sync.dma_start(pos1f, posb_d.ap())
            nc.sync.dma_start(neg1f, negb_d.ap())
            nc.sync.dma_start(featb, featb_d.ap())
            killb = consts.tile([128, NB], f32)
            nc.sync.dma_start(killb, killb_d.ap())

            # packed gather table (cols 34..39 uninitialized, never read)
            pt_v = pt_d.ap().rearrange("(b p) f -> p b f", p=128)
            nc.sync.dma_start(pt_v[:, :, 0:D], featb)
            nc.sync.dma_start(pt_v[:, :, D:D + 1],
                              sigb.rearrange("p (b o) -> p b o", o=1))
            nc.sync.dma_start(pt_v[:, :, D + 1:D + 2],
                              labbf.rearrange("p (b o) -> p b o", o=1))

            iota40f = consts.tile([128, 40], f32)
            nc.sync.dma_start(iota40f, iota40_d.ap())
            b1e7 = consts.tile([128, 1], f32)
            b1e8 = consts.tile([128, 1], f32)
            nc.vector.memset(b1e7, 1e-7)
            nc.vector.memset(b1e8, 1e-8)

            # preload ACT LUTs so the tail doesn't serialize table loads
            warm = consts.tile([128, 1], f32)
            for fn in (AF.Sign, AF.Sqrt, AF.Erf, AF.Ln):
                nc.scalar.activation(warm, b1e7, fn)

            vsum = consts.tile([128, NB], f32)
            accB = consts.tile([128, NB], f32)
            find_in = consts.tile([128, 8], f32)
            nc.vector.memset(find_in, NEG_INF)
            Gp = consts.tile([128, NB, 64], f32)
            Gn = consts.tile([128, NB, 64], f32)

            scr = scrp.tile([128, W], f32, tag="scr")
            dummy = scrp.tile([128, W], bf16, tag="dummy")
            V40 = scrp.tile([128, 40], f32, tag="V40")
            idx8 = scrp.tile([128, 8], u32, tag="idx8")

            # ================= block loop =================
            for b in range(NB):
                ob = _win_off(b)
                psumA = psA.tile([128, W], f32, tag="A")
                psumB = psB.tile([128, W], f32, tag="B")
                s_sb = sb.tile([128, W], f32, tag="s_sb")
                for o0, w0 in ((0, 512), (512, 512), (1024, 512), (1536, 256)):
                    nc.tensor.matmul(psumA[:, o0:o0 + w0],
                                     Ml7[:, 128 * b:128 * (b + 1)],
                                     Mm7[:, ob + o0:ob + o0 + w0],
                                     start=True, stop=True)
                for o0, w0 in ((0, 512), (512, 512), (1024, 512), (1536, 256)):
                    nc.tensor.matmul(psumB[:, o0:o0 + w0],
                                     Ml[:, 128 * b:128 * (b + 1)],
                                     Mm[:, ob + o0:ob + o0 + w0],
                                     start=True, stop=True)

                nc.scalar.activation(s_sb, psumA, AF.Copy)

                # sorted top-40: 5 max8 rounds, round 1 from PSUM
                nc.vector.max(out=V40[:, 0:8], in_=psumA)
                nc.vector.match_replace(out=scr, in_to_replace=V40[:, 0:8],
                                        in_values=psumA, imm_value=NEG_INF)
                for rnd in range(1, 5):
                    nc.vector.max(out=V40[:, 8 * rnd:8 * (rnd + 1)], in_=scr)
                    if rnd < 4:
                        nc.vector.match_replace(
                            out=scr, in_to_replace=V40[:, 8 * rnd:8 * (rnd + 1)],
                            in_values=scr, imm_value=NEG_INF)

                # threshold midpoint -> vsum; same-label count on ACT
                nc.vector.tensor_add(vsum[:, b:b + 1], V40[:, 35:36],
                                     V40[:, 36:37])
                nc.scalar.activation(dummy, psumB, AF.Sign, scale=-2.0,
                                     bias=vsum[:, b:b + 1],
                                     accum_out=accB[:, b:b + 1])

                # rank-select pos/neg values straight into find input
                scr40 = sb.tile([128, 40], f32, tag="scr40")
                nc.vector.scalar_tensor_tensor(
                    out=scr40, in0=iota40f, scalar=pos1f[:, b:b + 1], in1=V40,
                    op0=OP.is_equal, op1=OP.mult, accum_out=find_in[:, 0:1])
                scr40b = sb.tile([128, 40], f32, tag="scr40b")
                nc.vector.scalar_tensor_tensor(
                    out=scr40b, in0=iota40f, scalar=neg1f[:, b:b + 1], in1=V40,
                    op0=OP.is_equal, op1=OP.mult, accum_out=find_in[:, 1:2])

                # one pass recovers both column indices (local), add offset
                nc.vector.max_index(out=idx8, in_max=find_in, in_values=s_sb)
                jpn32 = sb.tile([128, 2], i32, tag="jpn")
                nc.vector.tensor_scalar(jpn32, idx8[:, 0:2],
                                        float(ob), 4095.0, op0=OP.add,
                                        op1=OP.min)

                # per-block indirect gather straight from SBUF offsets
                nc.gpsimd.indirect_dma_start(
                    out=Gp[:, b], out_offset=None, in_=pt_d.ap(),
                    in_offset=cbass.IndirectOffsetOnAxis(
                        ap=jpn32[:, 0:1], axis=0))
                nc.gpsimd.indirect_dma_start(
                    out=Gn[:, b], out_offset=None, in_=pt_d.ap(),
                    in_offset=cbass.IndirectOffsetOnAxis(
                        ap=jpn32[:, 1:2], axis=0))

            # ============== loss tail (split: blocks 0..23, 24..31) ==============
            import os
            dbg = bool(os.environ.get("KDBG"))

            def emit_tail(c0, c1):
                nb = c1 - c0
                cs = slice(c0, c1)
                sfx = f"_{c0}"
                cntf = consts.tile([128, nb], f32, name=f"cntf{sfx}")
                nc.vector.tensor_scalar(cntf, accB[:, cs], -0.5,
                                        W / 2.0 - 1.0,
                                        op0=OP.mult, op1=OP.add)

                prod = consts.tile([128, nb, D], f32, name=f"prod{sfx}")
                dAP = consts.tile([128, nb], f32, name=f"dAP{sfx}")
                dAN = consts.tile([128, nb], f32, name=f"dAN{sfx}")
                dPN = consts.tile([128, nb], f32, name=f"dPN{sfx}")
                GpF = Gp[:, cs, 0:D]
                GnF = Gn[:, cs, 0:D]
                for dst, u, v in ((dAP, featb[:, cs], GpF),
                                  (dAN, featb[:, cs], GnF), (dPN, GpF, GnF)):
                    nc.vector.tensor_mul(prod, u, v)
                    nc.vector.tensor_reduce(dst, prod, axis=AX.X, op=OP.add)

                vA = sigb[:, cs]
                vP = consts.tile([128, nb], f32, name=f"vP{sfx}")
                vN = consts.tile([128, nb], f32, name=f"vN{sfx}")
                labP = consts.tile([128, nb], f32, name=f"labP{sfx}")
                labN = consts.tile([128, nb], f32, name=f"labN{sfx}")
                nc.vector.tensor_copy(
                    vP, Gp[:, cs, D:D + 1].rearrange("p b o -> p (b o)"))
                nc.vector.tensor_copy(
                    vN, Gn[:, cs, D:D + 1].rearrange("p b o -> p (b o)"))
                nc.vector.tensor_copy(
                    labP, Gp[:, cs, D + 1:D + 2].rearrange("p b o -> p (b o)"))
                nc.vector.tensor_copy(
                    labN, Gn[:, cs, D + 1:D + 2].rearrange("p b o -> p (b o)"))

                t1 = consts.tile([128, nb], f32, name=f"t1{sfx}")
                t2 = consts.tile([128, nb], f32, name=f"t2{sfx}")
                t3 = consts.tile([128, nb], f32, name=f"t3{sfx}")
                w = consts.tile([128, nb], f32, name=f"w{sfx}")
                nc.vector.tensor_tensor(t1, labP, labbf[:, cs], op=OP.is_equal)
                nc.vector.tensor_tensor(t2, labN, labbf[:, cs],
                                        op=OP.not_equal)
                nc.vector.tensor_mul(w, t1, t2)
                nc.vector.tensor_scalar(t1, cntf, 0.5, None, op0=OP.is_ge)
                nc.vector.tensor_mul(w, w, t1)
                nc.vector.tensor_scalar(t1, cntf, K - 1.5, None, op0=OP.is_le)
                nc.vector.tensor_mul(w, w, t1)
                nc.vector.tensor_mul(w, w, killb[:, cs])

                # mu = D*(vP - vN) - 2*(dAP - dAN)     (dPP = dNN = 1)
                mu = consts.tile([128, nb], f32, name=f"mu{sfx}")
                nc.vector.tensor_sub(t1, vP, vN)
                nc.vector.tensor_sub(t2, dAP, dAN)
                nc.vector.tensor_scalar_mul(t1, t1, float(D))
                nc.vector.scalar_tensor_tensor(
                    out=mu, in0=t2, scalar=-2.0, in1=t1,
                    op0=OP.mult, op1=OP.add)

                # sum_d T = D*vX^2 + (4 + 2D*vA - 4*dAX)*vX + 2*vA (dXX=dAA=1)
                def sT(out, vX, dAX):
                    nc.vector.tensor_scalar(t1, vA, 2.0 * D, 4.0,
                                            op0=OP.mult, op1=OP.add)
                    nc.vector.scalar_tensor_tensor(
                        out=t1, in0=dAX, scalar=-4.0, in1=t1,
                        op0=OP.mult, op1=OP.add)
                    nc.vector.tensor_mul(t1, t1, vX)
                    nc.vector.scalar_tensor_tensor(
                        out=t1, in0=vA, scalar=2.0, in1=t1,
                        op0=OP.mult, op1=OP.add)
                    nc.vector.scalar_tensor_tensor(
                        out=out, in0=vX, scalar=float(D), in1=vX,
                        op0=OP.mult, op1=OP.mult)
                    nc.vector.tensor_add(out, out, t1)

                sigma2 = consts.tile([128, nb], f32, name=f"sigma2{sfx}")
                sT(t2, vP, dAP)
                sT(t3, vN, dAN)
                nc.vector.tensor_add(sigma2, t2, t3)
                nc.vector.tensor_mul(t1, vA, dPN)
                nc.vector.scalar_tensor_tensor(
                    out=sigma2, in0=t1, scalar=-4.0, in1=sigma2,
                    op0=OP.mult, op1=OP.add)
                nc.vector.tensor_scalar_mul(sigma2, sigma2, 2.0)
                nc.vector.tensor_scalar_max(sigma2, sigma2, 0.0)

                sig = consts.tile([128, nb], f32, name=f"sig{sfx}")
                nc.scalar.activation(sig, sigma2, AF.Sqrt, bias=b1e7)
                nc.vector.tensor_scalar(t1, sig, 1e-8, float(np.sqrt(2.0)),
                                        op0=OP.add, op1=OP.mult)
                nc.vector.reciprocal(t2, t1)
                nc.vector.tensor_mul(t1, mu, t2)
                probs = consts.tile([128, nb], f32, name=f"probs{sfx}")
                nc.scalar.activation(probs, t1, AF.Erf, scale=-1.0)
                nc.vector.tensor_scalar(probs, probs, 0.5, 0.5,
                                        op0=OP.mult, op1=OP.add)
                nll = consts.tile([128, nb], f32, name=f"nll{sfx}")
                nc.scalar.activation(nll, probs, AF.Ln, bias=b1e8)
                nc.vector.tensor_scalar_mul(nll, nll, -1.0)

                # kl = (D/2)/VP*(vA+vP+vN) + const - (D/2)*ln(vA*vP*vN)
                kl = consts.tile([128, nb], f32, name=f"kl{sfx}")
                nc.vector.tensor_add(t1, vA, vP)
                nc.vector.tensor_add(t1, t1, vN)
                nc.vector.tensor_mul(t2, vA, vP)
                nc.vector.tensor_mul(t2, t2, vN)
                lnv = consts.tile([128, nb], f32, name=f"lnv{sfx}")
                nc.scalar.activation(lnv, t2, AF.Ln)
                kconst = 3.0 * (0.5 / VAR_PRIOR - D / 2.0
                                + (D / 2.0) * float(np.log(VAR_PRIOR)))
                nc.vector.tensor_scalar(t1, t1, 0.5 * D / VAR_PRIOR, kconst,
                                        op0=OP.mult, op1=OP.add)
                nc.vector.scalar_tensor_tensor(
                    out=kl, in0=lnv, scalar=-0.5 * D, in1=t1,
                    op0=OP.mult, op1=OP.add)

                rows = (w, nll, probs, mu, sig, kl, vsum[:, cs],
                        accB[:, cs]) if dbg else \
                    (w, nll, probs, mu, sig, kl, cntf, cntf)
                for qi, rsrc in enumerate(rows):
                    dst = outv_d.ap()[:, qi * NB + c0:qi * NB + c1]
                    if qi in (0, 6, 7):
                        nc.sync.dma_start(dst, rsrc)
                    else:
                        ot = consts.tile([128, nb], f32, name=f"o{qi}{sfx}")
                        nc.vector.tensor_mul(ot, rsrc, w)
                        nc.sync.dma_start(dst, ot)

            emit_tail(0, NB)

    nc.compile()
    return nc


_prog = None


def _get_prog():
    global _prog
    if _prog is None:
        _prog = build_program()
    return _prog


def _bf(x):
    return x.astype(bfnp)


def _f(x):
    return x.astype(np.float32)


def _build_M(pts, lab):
    """Host-side bf16 triple-split M matrices [30, P] (lhs, mov)."""
    x = np.ascontiguousarray(pts.T).astype(np.float32)      # [3, P]
    xh = _bf(x)
    res = x - _f(xh)
    xm = _bf(res)
    xl = _bf(res - _f(xm))
    nsq = -(x * x)
    nqh = _bf(nsq)
    nqr = nsq - _f(nqh)
    nqm = _bf(nqr)
    nql = _bf(nqr - _f(nqm))
    x2, x2b, x2c = _bf(2.0 * _f(xh)), _bf(2.0 * _f(xm)), _bf(2.0 * _f(xl))
    ones = np.ones((3, P), dtype=bfnp)
    labf = lab.astype(np.float32)

    Ml = np.zeros((30, P), dtype=bfnp)
    Mm = np.zeros((30, P), dtype=bfnp)
    # label penalty rows first: exact 0 for same-label pairs
    Ml[0] = _bf(-CLAB * labf * labf)
    Mm[0] = ones[0]
    Ml[1] = _bf(labf)
    Mm[1] = _bf(2.0 * CLAB * labf)
    Ml[2] = ones[0]
    Mm[2] = _bf(-CLAB * labf * labf)
    # s rows (baseline ordering), shifted by 3
    Ml[3:6], Mm[3:6] = x2b, xm          # mm
    Ml[6:9], Mm[6:9] = x2, xl           # hl
    Ml[9:12], Mm[9:12] = x2c, xh        # lh
    Ml[12:15], Mm[12:15] = ones, nql    # ql
    Ml[15:18], Mm[15:18] = x2, xm       # hm
    Ml[18:21], Mm[18:21] = x2b, xh      # mh
    Ml[21:24], Mm[21:24] = ones, nqm    # qm
    for c in range(3):
        Ml[24 + 2 * c], Mm[24 + 2 * c] = x2[c], xh[c]       # hh
        Ml[25 + 2 * c], Mm[25 + 2 * c] = ones[0], nqh[c]    # qh
    return Ml, Mm


def per_core_inputs(feature, sigma, xyz, label, pos_idx, neg_idx, c):
    lo, hi = c * P, (c + 1) * P
    pts = xyz[lo:hi, 1:4].astype(np.float64)
    order = np.argsort(pts[:, 0], kind='stable')
    pts = pts[order]
    lab = label[lo:hi, 0].astype(np.int32)[order]
    sig = sigma[lo:hi, 0].astype(np.float32)[order]
    pos = pos_idx[lo:hi].astype(np.int32)[order]
    neg = neg_idx[lo:hi].astype(np.int32)[order]
    same = pos == neg
    neg = np.where(same, (neg + 1) % (K - 1), neg).astype(np.int32)
    kill = (1.0 - same).astype(np.float32)
    feat = feature[lo:hi].astype(np.float64)[order]
    featN = (feat / np.linalg.norm(feat, axis=1, keepdims=True)).astype(
        np.float32)
    Ml, Mm = _build_M(pts.astype(np.float32), lab)
    return {
        "Ml": Ml,
        "Mm": Mm,
        "Ml7": np.ascontiguousarray(Ml[3:30]),
        "Mm7": np.ascontiguousarray(Mm[3:30]),
        "labf": np.ascontiguousarray(lab.astype(np.float32)
                                     .reshape(NB, 128).T),
        "sigb": np.ascontiguousarray(sig.reshape(NB, 128).T),
        "pos1f": np.ascontiguousarray((pos + 1).astype(np.float32)
                                      .reshape(NB, 128).T),
        "neg1f": np.ascontiguousarray((neg + 1).astype(np.float32)
                                      .reshape(NB, 128).T),
        "iota40f": np.tile(np.arange(40, dtype=np.float32), (128, 1)),
        "featb": np.ascontiguousarray(
            featN.reshape(NB, 128, D).transpose(1, 0, 2)),
        "killb": np.ascontiguousarray(kill.reshape(NB, 128).T),
    }


def unpack_rows(res):
    return np.concatenate(
        [r["outv"].astype(np.float64).reshape(128, 8, NB)
         .transpose(1, 2, 0).reshape(8, P) for r in res.results], axis=1)


def finalize(rows):
    ws = max(rows[0].sum(), 1.0)
    nll_m, probs_m, mu_m, sig_m, kl_m = (rows[i].sum() / ws
                                         for i in range(1, 6))
    loss = nll_m + KL_SCALE * kl_m
    return (np.float32(loss), np.float32(probs_m), np.float32(mu_m),
            np.float32(sig_m))


def kernel(feature, sigma, xyz, label, pos_idx, neg_idx):
    nc = _get_prog()
    in_maps = [
        per_core_inputs(feature, sigma, xyz, label, pos_idx, neg_idx, c)
        for c in range(B)
    ]
    res = run_bass_kernel_spmd(nc, in_maps, core_ids=list(range(B)))
    return finalize(unpack_rows(res))
